# revision 1
# baseline (speedup 1.0000x reference)
"""Trainium2 Bass kernel for nn_ButterflyLayer2D (butterfly 2D CNN).

Strategy: pure data parallel over 8 NeuronCores (16 batch each), with the
per-core batch processed in 2 phases of 8 to fit SBUF.

All tensors are pre-arranged on the host (numpy) into DMA-friendly layouts:
  - activations live in SBUF as [128 = (w%2)*64 + c, (node, b, h, w//2)]
    so each 2x2-stride-2 per-node conv becomes 4 fp32r matmuls with K=128
    chunks: col-group q = output w-parity (tile_position (0, 64q)), x-chunks
    accumulate in PSUM. PSUM [128=(q,c_out), N] is evicted full-width by a
    single relu+bias op (alternating ScalarE/VectorE) directly into the next
    level's interleaved layout — zero data reshuffling anywhere on chip.
  - the input 4x4-patch conv uses the same trick with K=16 row-groups spread
    over 4 partition groups (one per b%4) for PE concurrency.
  - the final per-node dense is a [64,128] x [64,b] matmul; outputs are
    written as [128=(r,ou,ov), (ph,node,b)] and decoded on the host.
Weights are streamed from HBM in 8-node chunks through a recycled tile tag.
"""

import numpy as np
from contextlib import ExitStack

import concourse.bass as bass
import concourse.tile as tile
from concourse import bacc, mybir
from concourse.bass_utils import run_bass_kernel_spmd

F32 = mybir.dt.float32
F32R = mybir.dt.float32r
BF16 = mybir.dt.bfloat16
AF = mybir.ActivationFunctionType
ALU = mybir.AluOpType

B, IN, NLVL, KLVL, C = 128, 256, 6, 3, 64
TCOL = 1024               # psum tile columns
PBUFS = 4                 # psum tile bufs
NK, OU, OV = 8, 8, 8
NCORES = 8
BC = B // NCORES          # 16 per-core batch
PH = 1                    # phases per core
BG = BC // PH             # batch per phase
HALF = BG // 4            # input-conv b-subgroups per partition group
LVL_NODES = [4, 16, 64, 64, 64, 64]          # nodes per level
LVL_HIN = [64, 32, 16, 8, 4, 2]              # spatial H into each level
WGRP = 8                  # weight streaming chunk (nodes)


# ----------------------------------------------------------------------------
# host-side pre-arrangement
# ----------------------------------------------------------------------------

def _prep_weights(inputs):
    """Weights/biases blobs shared by all cores."""
    out = {}
    # input filter: lhsT [16=(p,q), 64], replicated at partition bases 0/32/64/96
    import ml_dtypes
    fin = inputs["in_filter"][:, :, 0, :].reshape(16, C).astype(np.float32)
    finr = np.zeros((128, C), np.float32)
    for g in range(4):
        finr[g * 32 : g * 32 + 16] = fin
    out["fin"] = finr.astype(ml_dtypes.bfloat16)
    out["bin"] = np.concatenate([inputs["in_bias"], inputs["in_bias"]]).reshape(
        128, 1
    ).astype(np.float32)

    for lvl in range(1, NLVL + 1):
        f = inputs[f"f{lvl}"].astype(np.float32)  # [n,n,2,2,C,C] (x,y,ci,co)
        n = f.shape[0]
        assert n == 2 ** min(lvl, KLVL)
        # lhsT per node: [(y*64+ci), (x*64+co)]
        w = f.transpose(0, 1, 3, 4, 2, 5).reshape(n * n, 2 * C, 2 * C)
        if lvl <= KLVL:
            # sibling-pair blob: per pair (u,2t)+(u,2t+1):
            # [(y,ci)=128, (x, coA|coB)=256] -> [128, pairs*256]
            wp = w.reshape(n * n // 2, 2, 2 * C, 2, C)  # [pair, s, (y,ci), x, co]
            wp = wp.transpose(2, 0, 3, 1, 4)            # [(y,ci), pair, x, s, co]
            out[f"w{lvl}"] = np.ascontiguousarray(wp).reshape(
                128, n * n * 128
            ).astype(ml_dtypes.bfloat16)
        else:
            # blob [128, nodes*128], free = (node, x*64+co)
            out[f"w{lvl}"] = np.ascontiguousarray(w.transpose(1, 0, 2)).reshape(
                128, n * n * 128
            ).astype(ml_dtypes.bfloat16)
        b = inputs[f"b{lvl}"].astype(np.float32).reshape(n * n, C)
        if lvl < NLVL:
            # [128, nodes]: rows (q,c) with bias duplicated across q
            bb = np.concatenate([b, b], axis=1)  # [nodes, 128]
            out[f"b{lvl}"] = np.ascontiguousarray(bb.T)
        else:
            # lvl6 node-pair scheme: psum rows = (cA, cB) for pair (2k, 2k+1)
            bb = b.reshape(n * n // 2, 2 * C)  # [pairs, (cA,cB)]
            out[f"b{lvl}"] = np.ascontiguousarray(bb.T)  # [128, 32]
    # dense: lhsT per node [64=c, 128=(r, ou*8+ov)]
    wd = inputs["Wd"].astype(np.float32).reshape(NK * NK, 2, C, OU * OV)
    wd = wd.transpose(2, 0, 1, 3).reshape(C, NK * NK * 2 * OU * OV)
    out["wd"] = np.ascontiguousarray(wd).astype(ml_dtypes.bfloat16)
    return out


def _prep_input(in_data_core):
    """Per-core input blob: [64 = (b%4)*16 + (i%4)*4 + (j%4),
    (ph, b//4%2, x=i//4, y4=j//4)] packed (no zero rows)."""
    ind = in_data_core[:, :, :, 0]  # [16, 256, 256]
    a = ind.reshape(PH, HALF, 4, 64, 4, 64, 4)  # [ph, half, g, x, p, y4, q]
    a = a.transpose(2, 4, 6, 0, 1, 3, 5)        # [g, p, q, ph, half, x, y4]
    import ml_dtypes
    return np.ascontiguousarray(a).reshape(64, PH * HALF * 64 * 64).astype(ml_dtypes.bfloat16)


def _decode_output(t2_core):
    """t2 [128=(r,ou,ov), (ph, node, bl)] -> [16, 64, 64, 2]."""
    t = t2_core.reshape(2, OU, OV, PH, NK, NK, BG)  # r,ou,ov,ph,u,v,bl
    t = t.transpose(3, 6, 4, 1, 5, 2, 0)            # ph,bl,u,ou,v,ov,r
    return np.ascontiguousarray(t).reshape(BC, NK * OU, NK * OV, 2)


# ----------------------------------------------------------------------------
# device kernel
# ----------------------------------------------------------------------------

def _build_kernel(reps=1, xouter=True):
    nc = bacc.Bacc(None, target_bir_lowering=False)
    p = {}
    p["a0"] = nc.declare_dram_parameter("a0", [64, PH * HALF * 64 * 64], BF16, isOutput=False)
    p["fin"] = nc.declare_dram_parameter("fin", [128, C], BF16, isOutput=False)
    p["bin"] = nc.declare_dram_parameter("bin", [128, 1], F32, isOutput=False)
    for lvl in range(1, NLVL + 1):
        n2 = LVL_NODES[lvl - 1]
        p[f"w{lvl}"] = nc.declare_dram_parameter(f"w{lvl}", [128, n2 * 128], BF16, isOutput=False)
        bcols = n2 if lvl < NLVL else n2 // 2
        p[f"b{lvl}"] = nc.declare_dram_parameter(f"b{lvl}", [128, bcols], F32, isOutput=False)
    p["wd"] = nc.declare_dram_parameter("wd", [64, NK * NK * 128], BF16, isOutput=False)
    t2 = nc.declare_dram_parameter("t2", [128, PH * NK * NK * BG], F32, isOutput=True)

    evict_ctr = [0]

    def evict(out_ap, psum_ap, bias_ap):
        """relu(psum + bias) -> sbuf, alternating engines to split the load."""
        evict_ctr[0] += 1
        if evict_ctr[0] % 2 == 0:
            nc.scalar.activation(out_ap, psum_ap, AF.Relu, bias=bias_ap)
        else:
            nc.vector.tensor_scalar(out_ap, psum_ap, bias_ap, 0.0,
                                    op0=ALU.add, op1=ALU.max)

    with tile.TileContext(nc) as tc, ExitStack() as ctx:
        const = ctx.enter_context(tc.tile_pool(name="const", bufs=1))
        wpool = ctx.enter_context(tc.tile_pool(name="wts", bufs=4))
        apool = ctx.enter_context(tc.tile_pool(name="acts", bufs=1))
        inpool = ctx.enter_context(tc.tile_pool(name="inp", bufs=1))
        fpool = ctx.enter_context(tc.tile_pool(name="feat", bufs=2))
        ppool = ctx.enter_context(tc.tile_pool(name="ps", bufs=PBUFS, space="PSUM"))
        spool = ppool

        # constants: input filter, biases (all small, loaded once)
        fin_t = const.tile([128, C], BF16)
        nc.sync.dma_start(fin_t[:], p["fin"][:])
        bin_t = const.tile([128, 1], F32)
        nc.sync.dma_start(bin_t[:], p["bin"][:])
        bias_t = {}
        for lvl in range(1, NLVL + 1):
            bcols = LVL_NODES[lvl - 1] if lvl < NLVL else LVL_NODES[lvl - 1] // 2
            bias_t[lvl] = const.tile([128, bcols], F32, tag=f"bias{lvl}", name=f"bias{lvl}")
            nc.sync.dma_start(bias_t[lvl][:], p[f"b{lvl}"][:])

        for phx in range(reps * PH):
            ph = phx % PH
            # ---------------- input staging ----------------
            a0s = inpool.tile([128, HALF * 64 * 64], BF16, tag="a0s", name=f"a0s{phx}")
            for g in range(4):
                nc.sync.dma_start(
                    a0s[g * 32 : g * 32 + 16, :],
                    p["a0"][g * 16 : (g + 1) * 16,
                            ph * HALF * 64 * 64 : (ph + 1) * HALF * 64 * 64],
                )
            a0v = a0s[:].rearrange("p (h x y) -> p h x y", h=HALF, x=64)

            # ---------------- input conv ----------------
            # X slab: [128=(y%2,c), (bl, x, y2)]  (bl=8, x=64, y2=32)
            X = apool.tile([128, BG * 64 * 32], BF16, tag="s0", name=f"x{phx}")
            Xv = X[:].rearrange("p (b h w) -> p b h w", b=BG, h=64)
            for bl in range(BG):
                g, half = bl % 4, bl // 4
                for xh in range(2048 // TCOL):
                    pt = ppool.tile([128, TCOL], F32, tag="ps",
                                    padded_shape=[128, TCOL],
                                    name=f"pin{phx}_{bl}_{xh}")
                    for sub in range(TCOL // 512):
                        xq = xh * (TCOL // 512) + sub
                        for q in (0, 1):
                            rhs = a0v[g * 32 : g * 32 + 16, half,
                                      xq * 16 : (xq + 1) * 16, q::2]
                            nc.tensor.matmul(
                                pt[q * 64 : (q + 1) * 64,
                                   sub * 512 : (sub + 1) * 512],
                                fin_t[g * 32 : g * 32 + 16, :],
                                rhs,
                                start=True, stop=True,
                                tile_position=(g * 32, q * 64),
                            )
                    evict(Xv[:, bl, xh * (TCOL // 32) : (xh + 1) * (TCOL // 32), :], pt[:], bin_t[:, 0:1])

            # ---------------- levels 1..5 (q-scheme) ----------------
            cur = X          # slab with free = (node, bl, h, w2)
            cur_nodes = 1
            tags = ["s1", "s0", "s1", "s0", "s1"]
            for lvl in range(1, 6):
                n2 = LVL_NODES[lvl - 1]
                grid = int(np.sqrt(n2))
                Hin = LVL_HIN[lvl - 1]
                W2in = Hin // 2
                Ho, W2o = Hin // 2, W2in // 2
                ncols_out = BG * Ho * W2o
                nxt = apool.tile([128, n2 * ncols_out], BF16,
                                 tag=tags[lvl - 1], name=f"a{lvl}_{phx}")
                curv = cur[:].rearrange("p (n b h w) -> p n b h w",
                                        n=cur_nodes, b=BG, h=Hin)
                nxtv = nxt[:].rearrange("p (n b h w) -> p n b h w",
                                        n=n2, b=BG, h=Ho)
                pgrid = int(np.sqrt(cur_nodes))
                if lvl <= KLVL:
                    # sibling-pair scheme: M=128=(coA,coB), shared parent rhs
                    Wo = W2in          # output width = rhs w-count
                    npairs = n2 // 2
                    # block = (bper b, hper h, all Wo) == 1024 cols (2 banks)
                    hper = min(Ho, TCOL // Wo)
                    bper = min(BG, max(1, TCOL // (Wo * hper)))
                    ncol = bper * hper * Wo
                    PGRP = 4           # pairs per weight DMA chunk
                    for g0 in range(0, npairs, PGRP):
                        gn = min(PGRP, npairs - g0)
                        wlt = wpool.tile([128, PGRP * 256], BF16, tag="wch",
                                         name=f"w{lvl}_{phx}_{g0}")
                        nc.sync.dma_start(
                            wlt[:, : gn * 256],
                            p[f"w{lvl}"][:, g0 * 256 : (g0 + gn) * 256],
                        )
                        for pr in range(g0, g0 + gn):
                            u, t = pr // (grid // 2), pr % (grid // 2)
                            nA = u * grid + 2 * t
                            nB = nA + 1
                            lp_ = pr - g0
                            pnode = (u // 2) * pgrid + t
                            # sub-splitting along b (or h) into 512-col chunks
                            nsub = ncol // 512
                            if bper >= nsub:
                                sb, sh = bper // nsub, hper
                            else:
                                sb, sh = 1, hper // (nsub // max(1, bper))
                            hsubs = hper // sh
                            for bs in range(0, BG, bper):
                                for h0 in range(0, Ho, hper):
                                    pt = ppool.tile(
                                        [128, ncol], F32, tag="ps",
                                        padded_shape=[128, TCOL],
                                        name=f"p{lvl}_{phx}_{pr}_{bs}_{h0}")
                                    for sub in range(nsub):
                                        b1 = bs + (sub // hsubs) * sb
                                        h1 = h0 + (sub % hsubs) * sh
                                        for x in (0, 1):
                                            rhs = curv[:, pnode, b1 : b1 + sb,
                                                       2 * h1 + x : 2 * (h1 + sh) : 2,
                                                       :]
                                            nc.tensor.matmul(
                                                pt[:, sub * 512 : (sub + 1) * 512],
                                                wlt[:, lp_ * 256 + x * 128 :
                                                    lp_ * 256 + (x + 1) * 128],
                                                rhs,
                                                start=(x == 0), stop=(x == 1),
                                            )
                                    for shalf, node in ((0, nA), (1, nB)):
                                        ptv = pt[shalf * 64 : (shalf + 1) * 64, :] \
                                            .rearrange("c (b h w) -> c b h w",
                                                       b=bper, h=hper)
                                        bias_ap = bias_t[lvl][
                                            shalf * 64 : (shalf + 1) * 64,
                                            node : node + 1]
                                        for par in (0, 1):
                                            evict(
                                                nxtv[par * 64 : (par + 1) * 64,
                                                     node, bs : bs + bper,
                                                     h0 : h0 + hper, :],
                                                ptv[:, :, :, par::2],
                                                bias_ap,
                                            )
                else:
                    # q-scheme (deep levels)
                    nblk = max(1, ncols_out // 512)
                    bper = BG // nblk
                    ncol = bper * Ho * W2o
                    for g0 in range(0, n2, WGRP):
                        gn = min(WGRP, n2 - g0)
                        wlt = wpool.tile([128, WGRP * 128], BF16, tag="wch",
                                         name=f"w{lvl}_{phx}_{g0}")
                        nc.sync.dma_start(
                            wlt[:, : gn * 128],
                            p[f"w{lvl}"][:, g0 * 128 : (g0 + gn) * 128],
                        )
                        for node in range(g0, g0 + gn):
                            ln = node - g0
                            pnode = node
                            for blk in range(nblk):
                                bs = blk * bper
                                pt = ppool.tile([128, ncol], F32, tag="ps",
                                                padded_shape=[128, TCOL],
                                                name=f"p{lvl}_{phx}_{node}_{blk}")
                                qx = [(x, q) for x in (0, 1) for q in (0, 1)] \
                                    if xouter else \
                                    [(x, q) for q in (0, 1) for x in (0, 1)]
                                for x, q in qx:
                                    rhs = curv[:, pnode, bs : bs + bper, x::2, q::2]
                                    nc.tensor.matmul(
                                        pt[q * 64 : (q + 1) * 64, :],
                                        wlt[:, ln * 128 + x * 64 :
                                            ln * 128 + (x + 1) * 64],
                                        rhs,
                                        start=(x == 0), stop=(x == 1),
                                        skip_group_check=xouter,
                                        tile_position=(0, q * 64),
                                    )
                                evict(
                                    nxtv[:, node, bs : bs + bper, :, :],
                                    pt[:],
                                    bias_t[lvl][:, node : node + 1],
                                )
                cur = nxt
                cur_nodes = n2

            # ---------------- level 6 (node pairs, M=64) ----------------
            # cur: [128, (n=64, bl, h=2, w2=1)] ; feats F [64=c, (node, bl)]
            F = fpool.tile([64, NK * NK * BG], BF16, tag="feats", name=f"f{phx}")
            Fv = F[:].rearrange("c (n b) -> c n b", n=NK * NK)
            curv = cur[:].rearrange("p (n b h w) -> p n b h w", n=64, b=BG, h=2)
            for g0 in range(0, 64, WGRP):
                w6t = wpool.tile([128, WGRP * 128], BF16, tag="wch",
                                 name=f"w6_{phx}_{g0}")
                nc.sync.dma_start(
                    w6t[:], p["w6"][:, g0 * 128 : (g0 + WGRP) * 128]
                )
                for pr in range(g0 // 2, (g0 + WGRP) // 2):
                    nA, nB = 2 * pr, 2 * pr + 1
                    pt = spool.tile([128, BG], F32, tag="ps", padded_shape=[128, TCOL],
                                    name=f"p6_{phx}_{pr}")
                    hx = [(h_, x_) for x_ in (0, 1) for h_ in (0, 1)] \
                        if xouter else \
                        [(h_, x_) for h_ in (0, 1) for x_ in (0, 1)]
                    for half, x in hx:
                        node = nA if half == 0 else nB
                        ln = node - g0
                        rhs = curv[:, node, :, x, 0]
                        nc.tensor.matmul(
                            pt[half * 64 : (half + 1) * 64, :],
                            w6t[:, ln * 128 + x * 64 :
                                ln * 128 + (x + 1) * 64],
                            rhs,
                            start=(x == 0), stop=(x == 1),
                            skip_group_check=xouter,
                            tile_position=(0, half * 64),
                        )
                    bias_ap = bias_t[6][:, pr : pr + 1]
                    evict_ctr[0] += 1
                    if evict_ctr[0] % 2 == 0:
                        nc.scalar.activation(Fv[0:64, nA, :], pt[0:64, :], AF.Relu,
                                             bias=bias_ap[0:64, :])
                        nc.scalar.activation(Fv[0:64, nB, :], pt[64:128, :], AF.Relu,
                                             bias=bias_ap[64:128, :])
                    else:
                        nc.vector.tensor_scalar(Fv[0:64, nA, :], pt[0:64, :],
                                                bias_ap[0:64, :], 0.0,
                                                op0=ALU.add, op1=ALU.max)
                        nc.vector.tensor_scalar(Fv[0:64, nB, :], pt[64:128, :],
                                                bias_ap[64:128, :], 0.0,
                                                op0=ALU.add, op1=ALU.max)

            # ---------------- dense ----------------
            t2s = fpool.tile([128, NK * NK * BG], F32, tag="t2s", name=f"t2s{phx}")
            t2sv = t2s[:].rearrange("m (n b) -> m n b", n=NK * NK)
            for g0 in range(0, 64, WGRP):
                wdt = wpool.tile([64, WGRP * 128], BF16, tag="wdch",
                                 name=f"wd_{phx}_{g0}")
                nc.sync.dma_start(
                    wdt[:], p["wd"][:, g0 * 128 : (g0 + WGRP) * 128]
                )
                for node in range(g0, g0 + WGRP):
                    ln = node - g0
                    pt = spool.tile([128, BG], F32, tag="ps", padded_shape=[128, TCOL],
                                    name=f"pd_{phx}_{node}")
                    nc.tensor.matmul(
                        pt[:],
                        wdt[:, ln * 128 : (ln + 1) * 128],
                        Fv[:, node, :],
                        start=True, stop=True,
                    )
                    evict_ctr[0] += 1
                    if evict_ctr[0] % 2 == 0:
                        nc.scalar.copy(t2sv[:, node, :], pt[:])
                    else:
                        nc.vector.tensor_copy(t2sv[:, node, :], pt[:])
            nc.sync.dma_start(
                t2[:, ph * NK * NK * BG : (ph + 1) * NK * NK * BG], t2s[:]
            )
    nc.compile()
    return nc


# ----------------------------------------------------------------------------
# entry point
# ----------------------------------------------------------------------------

def kernel(**inputs):
    inputs = {k: np.asarray(v) for k, v in inputs.items()}
    wblobs = _prep_weights(inputs)
    nc = _build_kernel()
    in_maps = []
    for c in range(NCORES):
        m = dict(wblobs)
        m["a0"] = _prep_input(inputs["in_data"][c * BC : (c + 1) * BC])
        in_maps.append(m)
    res = run_bass_kernel_spmd(nc, in_maps, list(range(NCORES)))
    outs = [_decode_output(res.results[c]["t2"]) for c in range(NCORES)]
    return np.concatenate(outs, axis=0).astype(np.float32)


if __name__ == "__main__":
    import reference as ref

    inputs = {k: np.asarray(v) for k, v in ref.setup_inputs().items()}
    expected = np.asarray(ref.reference(**inputs))
    actual = kernel(**inputs)
    err = np.abs(actual - expected).max()
    rel = err / np.abs(expected).max()
    print("absmax:", err, "rel:", rel)



# revision 4
# speedup vs baseline: 1.6844x; 1.6844x over previous
"""Trainium2 Bass kernel for nn_ButterflyLayer2D (butterfly 2D CNN).

Strategy: pure data parallel over 8 NeuronCores (16 batch each).

Layouts (per core, bf16 activations):
  - conv-level inputs live in SBUF as [128 = (w%2)*64 + c, (node, b, h, w2)]
    so each 2x2-stride-2 per-node conv is computed with the q-scheme:
    output-w-parity q lands in psum partition half q via col-masked M=64
    matmuls at tile_position (0, 64q) (the two q matmuls run concurrently
    on the PE), x (h-parity) accumulates in PSUM.  K = (y, ci) = 128.
  - psum tiles then map 1:1 onto the next level's layout: partition
    (q, co) -> (w%2, c), cols (b, ho, w2o) -> (b, h, w2).  Every eviction
    is a single full-width 128-partition op with contiguous reads and
    writes (relu+bias fused via ScalarE activation / VectorE tensor_scalar,
    alternating engines).
  - deep levels (4, 5) batch many nodes into one [128, 1024] psum tile and
    evict with a 2-pass batched op (tensor_tensor add with a broadcast
    per-node bias AP, then relu) to amortize per-op overheads.
  - level 6 packs node pairs: psum [128=(s, c), (pair, b)]; dense reads the
    resulting F [128=(s, c), (pair, b)] with row-tiled K=64 matmuls
    (tile_position (64s, 0)) writing [128=(r,ou,ov), (pair, b)] per s.
  - the input 4x4-patch conv uses K=16 row-groups spread over 4 partition
    groups (one per b%4) x 2 col groups for 8-way PE tile concurrency.
  - ~20 warm-up matmuls on a memset tile run at t=0 (concurrent with input
    DMA) so the PE HAM clock-gate reaches 8/8 before the real work starts.
Weights are streamed from HBM in chunks through recycled tile tags; DMAs
are issued in consumption order with the input staged first.
"""

import numpy as np
from contextlib import ExitStack

import concourse.bass as bass
import concourse.tile as tile
from concourse import bacc, mybir
from concourse.bass_utils import run_bass_kernel_spmd

F32 = mybir.dt.float32
BF16 = mybir.dt.bfloat16
AF = mybir.ActivationFunctionType
ALU = mybir.AluOpType

B, IN, NLVL, KLVL, C = 128, 256, 6, 3, 64
TCOL = 1024               # psum tile columns
PBUFS = 4                 # psum tile bufs
NK, OU, OV = 8, 8, 8
NCORES = 8
BC = B // NCORES          # 16 per-core batch
PH = 1                    # phases per core
BG = BC // PH             # batch per phase
HALF = BG // 4            # input-conv b-subgroups per partition group
LVL_NODES = [4, 16, 64, 64, 64, 64]          # nodes per level
LVL_HIN = [64, 32, 16, 8, 4, 2]              # spatial H into each level
NWARM = 20                # HAM warm-up matmuls


# ----------------------------------------------------------------------------
# host-side pre-arrangement
# ----------------------------------------------------------------------------

def _prep_weights(inputs):
    """Weights/biases blobs shared by all cores."""
    import ml_dtypes
    out = {}
    # input filter: lhsT [16=(p,q), 64], replicated at partition bases 0/32/64/96
    fin = inputs["in_filter"][:, :, 0, :].reshape(16, C).astype(np.float32)
    finr = np.zeros((128, C), np.float32)
    for g in range(4):
        finr[g * 32 : g * 32 + 16] = fin
    out["fin"] = finr.astype(ml_dtypes.bfloat16)
    out["bin"] = np.concatenate([inputs["in_bias"], inputs["in_bias"]]).reshape(
        128, 1
    ).astype(np.float32)

    for lvl in range(1, NLVL + 1):
        f = inputs[f"f{lvl}"].astype(np.float32)  # [n,n,2,2,C,C] (x,y,ci,co)
        n = f.shape[0]
        n2 = n * n
        # per node lhsT [(y*64+ci), (x*64+co)], node-major blob
        w = f.transpose(0, 1, 3, 4, 2, 5).reshape(n2, 2 * C, 2 * C)
        out[f"w{lvl}"] = np.ascontiguousarray(w.transpose(1, 0, 2)).reshape(
            128, n2 * 128
        ).astype(ml_dtypes.bfloat16)
        b = inputs[f"b{lvl}"].astype(np.float32).reshape(n2, C)
        if lvl < NLVL:
            # [128, nodes]: rows (q,c) with bias duplicated across q
            bb = np.concatenate([b, b], axis=1)  # [nodes, 128]
            out[f"b{lvl}"] = np.ascontiguousarray(bb.T)
        else:
            # level-6 bias broadcast blob [128=(s,c), (pair, b)]
            bb = b.reshape(n2 // 2, 2, C)            # [pair, s, c]
            bb = bb.transpose(1, 2, 0)               # [s, c, pair]
            bb = np.repeat(bb.reshape(128, n2 // 2, 1), BG, axis=2)
            out["b6bc"] = np.ascontiguousarray(
                bb.reshape(128, n2 // 2 * BG)
            ).astype(ml_dtypes.bfloat16)
    # dense: blob [128=(s*64+c), (pair, r*64+ou*8+ov)]
    wd = inputs["Wd"].astype(np.float32).reshape(NK * NK, 2, C, OU * OV)
    wd = wd.reshape(NK * NK // 2, 2, 2, C, OU * OV)   # [pair, s, r, c, k]
    wd = wd.transpose(1, 3, 0, 2, 4)                  # [s, c, pair, r, k]
    out["wd"] = np.ascontiguousarray(wd.reshape(128, NK * NK // 2 * 128)).astype(
        ml_dtypes.bfloat16
    )
    return out


def _prep_input(in_data_core):
    """Per-core input blob: [64 = (b%4)*16 + (i%4)*4 + (j%4),
    (ph, b//4%2, x=i//4, y4=j//4)] packed (no zero rows)."""
    import ml_dtypes
    ind = in_data_core[:, :, :, 0]  # [16, 256, 256]
    a = ind.reshape(PH, HALF, 4, 64, 4, 64, 4)  # [ph, half, g, x, p, y4, q]
    a = a.transpose(2, 4, 6, 0, 1, 3, 5)        # [g, p, q, ph, half, x, y4]
    return np.ascontiguousarray(a).reshape(64, PH * HALF * 64 * 64).astype(ml_dtypes.bfloat16)


def _decode_output(t2_core):
    """t2 [128=(r,ou,ov), (ph, node, bl)] -> [16, 64, 64, 2]."""
    t = t2_core.reshape(2, OU, OV, PH, NK, NK, BG)  # r,ou,ov,ph,u,v,bl
    t = t.transpose(3, 6, 4, 1, 5, 2, 0)            # ph,bl,u,ou,v,ov,r
    return np.ascontiguousarray(t).reshape(BC, NK * OU, NK * OV, 2)


# ----------------------------------------------------------------------------
# device kernel
# ----------------------------------------------------------------------------

def _build_kernel():
    nc = bacc.Bacc(None, target_bir_lowering=False)
    p = {}
    p["a0"] = nc.declare_dram_parameter("a0", [64, PH * HALF * 64 * 64], BF16, isOutput=False)
    p["fin"] = nc.declare_dram_parameter("fin", [128, C], BF16, isOutput=False)
    p["bin"] = nc.declare_dram_parameter("bin", [128, 1], F32, isOutput=False)
    for lvl in range(1, NLVL + 1):
        n2 = LVL_NODES[lvl - 1]
        p[f"w{lvl}"] = nc.declare_dram_parameter(f"w{lvl}", [128, n2 * 128], BF16, isOutput=False)
        if lvl < NLVL:
            p[f"b{lvl}"] = nc.declare_dram_parameter(f"b{lvl}", [128, n2], F32, isOutput=False)
    p["b6bc"] = nc.declare_dram_parameter("b6bc", [128, 32 * BG], BF16, isOutput=False)
    p["wd"] = nc.declare_dram_parameter("wd", [128, 32 * 128], BF16, isOutput=False)
    t2 = nc.declare_dram_parameter("t2", [128, PH * NK * NK * BG], F32, isOutput=True)

    evict_ctr = [0]

    def evict(out_ap, psum_ap, bias_ap):
        """relu(psum + bias) -> sbuf, alternating engines to split the load."""
        evict_ctr[0] += 1
        if evict_ctr[0] % 2 == 0:
            nc.scalar.activation(out_ap, psum_ap, AF.Relu, bias=bias_ap)
        else:
            nc.vector.tensor_scalar(out_ap, psum_ap, bias_ap, 0.0,
                                    op0=ALU.add, op1=ALU.max)

    with tile.TileContext(nc) as tc, ExitStack() as ctx:
        const = ctx.enter_context(tc.tile_pool(name="const", bufs=1))
        wpool = ctx.enter_context(tc.tile_pool(name="wts", bufs=3))
        apool = ctx.enter_context(tc.tile_pool(name="acts", bufs=1))
        inpool = ctx.enter_context(tc.tile_pool(name="inp", bufs=1))
        fpool = ctx.enter_context(tc.tile_pool(name="feat", bufs=2))
        tpool = ctx.enter_context(tc.tile_pool(name="tmp", bufs=2))
        ppool = ctx.enter_context(tc.tile_pool(name="ps", bufs=PBUFS, space="PSUM"))

        # ------------- HAM warm-up (runs while input DMA streams) -------------
        wm = const.tile([128, 256], BF16, name="warm")
        nc.vector.memset(wm[:], 0.0)
        for i in range(NWARM):
            ptw = ppool.tile([128, 256], F32, tag="ps", padded_shape=[128, TCOL],
                             name=f"warm{i}")
            nc.tensor.matmul(ptw[:], wm[:, 0:128], wm[:],
                             start=True, stop=True)

        # ------------- input + constant DMAs (consumption order) -------------
        fin_t = const.tile([128, C], BF16)
        nc.sync.dma_start(fin_t[:], p["fin"][:])
        bin_t = const.tile([128, 1], F32)
        nc.sync.dma_start(bin_t[:], p["bin"][:])
        a0s = inpool.tile([128, PH * HALF * 64 * 64], BF16, tag="a0s", name="a0s")
        for g in range(4):
            nc.sync.dma_start(
                a0s[g * 32 : g * 32 + 16, :],
                p["a0"][g * 16 : (g + 1) * 16, :],
            )
        bias_t = {}
        for lvl in range(1, NLVL):
            bias_t[lvl] = const.tile([128, LVL_NODES[lvl - 1]], F32,
                                     tag=f"bias{lvl}", name=f"bias{lvl}")
            nc.sync.dma_start(bias_t[lvl][:], p[f"b{lvl}"][:])
        b6bc_t = const.tile([128, 32 * BG], BF16, name="b6bc")
        nc.sync.dma_start(b6bc_t[:], p["b6bc"][:])

        ph = 0
        a0v = a0s[:].rearrange("p (h x y) -> p h x y", h=HALF, x=64)

        # ---------------- input conv ----------------
        # X slab: [128=(y%2,c), (b, h=64, w2=32)]
        X = apool.tile([128, BG * 64 * 32], BF16, tag="s0", name="x0")
        Xv = X[:].rearrange("p (b h w) -> p b h w", b=BG, h=64)
        for bl in range(BG):
            g, half = bl % 4, bl // 4
            for xh in range(2):
                pt = ppool.tile([128, TCOL], F32, tag="ps",
                                padded_shape=[128, TCOL],
                                name=f"pin{bl}_{xh}")
                for sub in range(2):
                    xq = xh * 2 + sub
                    for q in (0, 1):
                        rhs = a0v[g * 32 : g * 32 + 16, half,
                                  xq * 16 : (xq + 1) * 16, q::2]
                        nc.tensor.matmul(
                            pt[q * 64 : (q + 1) * 64,
                               sub * 512 : (sub + 1) * 512],
                            fin_t[g * 32 : g * 32 + 16, :],
                            rhs,
                            start=True, stop=True,
                            tile_position=(g * 32, q * 64),
                        )
                evict(Xv[:, bl, xh * 32 : (xh + 1) * 32, :], pt[:], bin_t[:, 0:1])

        # ---------------- levels 1..3 (q-scheme, per-node psum) --------------
        cur, cur_nodes = X, 1
        tags = ["s1", "s0", "s1"]
        for lvl in (1, 2, 3):
            n2 = LVL_NODES[lvl - 1]
            grid = int(np.sqrt(n2))
            pgrid = int(np.sqrt(cur_nodes))
            Hin = LVL_HIN[lvl - 1]
            W2in = Hin // 2
            Ho, Ko = Hin // 2, W2in // 2      # psum cols per b = Ho*Ko
            ncolb = Ho * Ko
            bper = min(BG, TCOL // ncolb)
            nsub = (bper * ncolb) // 512       # 512-col chunks per tile
            bsub = bper // nsub
            nxt = apool.tile([128, n2 * BG * ncolb], BF16,
                             tag=tags[lvl - 1], name=f"a{lvl}")
            curv = cur[:].rearrange("p (n b h w) -> p n b h w",
                                    n=cur_nodes, b=BG, h=Hin)
            nxtv = nxt[:].rearrange("p (n b h w) -> p n b h w",
                                    n=n2, b=BG, h=Ho)
            # stream this level's weights in one or two chunks
            wchunk = min(n2, 16)
            for g0 in range(0, n2, wchunk):
                wlt = wpool.tile([128, 16 * 128], BF16, tag="wch",
                                 name=f"w{lvl}_{g0}")
                nc.sync.dma_start(
                    wlt[:, : wchunk * 128],
                    p[f"w{lvl}"][:, g0 * 128 : (g0 + wchunk) * 128],
                )
                for node in range(g0, g0 + wchunk):
                    u, v = node // grid, node % grid
                    pn = (u // 2) * pgrid + (v // 2)
                    ln = node - g0
                    for bs in range(0, BG, bper):
                        pt = ppool.tile([128, bper * ncolb], F32, tag="ps",
                                        padded_shape=[128, TCOL],
                                        name=f"p{lvl}_{node}_{bs}")
                        for x in (0, 1):
                            for q in (0, 1):
                                for sb in range(nsub):
                                    b1 = bs + sb * bsub
                                    rhs = curv[:, pn, b1 : b1 + bsub,
                                               x::2, q::2]
                                    nc.tensor.matmul(
                                        pt[q * 64 : (q + 1) * 64,
                                           sb * 512 : (sb + 1) * 512],
                                        wlt[:, ln * 128 + x * 64 :
                                            ln * 128 + (x + 1) * 64],
                                        rhs,
                                        start=(x == 0), stop=(x == 1),
                                        skip_group_check=True,
                                        tile_position=(0, q * 64),
                                    )
                        evict(
                            nxtv[:, node, bs : bs + bper, :, :],
                            pt[:],
                            bias_t[lvl][:, node : node + 1],
                        )
            cur, cur_nodes = nxt, n2

        # ---------------- levels 4..5 (q-scheme, node-batched psum) ----------
        for lvl in (4, 5):
            n2 = 64
            Hin = LVL_HIN[lvl - 1]
            W2in = Hin // 2
            Ho, Ko = Hin // 2, W2in // 2
            ncoln = BG * Ho * max(Ko, 1)       # cols per node (Ko>=1)
            gper = min(TCOL // ncoln, 16)      # nodes per psum tile
            nxt = apool.tile([128, n2 * ncoln], BF16,
                             tag=("s0" if lvl == 4 else "s1"), name=f"a{lvl}")
            curv = cur[:].rearrange("p (n b h w) -> p n b h w",
                                    n=64, b=BG, h=Hin)
            nxtv = nxt[:].rearrange("p (n c) -> p n c", n=n2)
            for g0 in range(0, n2, 16):
                wlt = wpool.tile([128, 16 * 128], BF16, tag="wch",
                                 name=f"w{lvl}_{g0}")
                nc.sync.dma_start(
                    wlt[:], p[f"w{lvl}"][:, g0 * 128 : (g0 + 16) * 128]
                )
                for t0 in range(g0, g0 + 16, gper):
                    pt = ppool.tile([128, gper * ncoln], F32, tag="ps",
                                    padded_shape=[128, TCOL],
                                    name=f"p{lvl}_{t0}")
                    for node in range(t0, t0 + gper):
                        ln, lt = node - g0, node - t0
                        for x in (0, 1):
                            for q in (0, 1):
                                rhs = curv[:, node, :, x::2, q::2]
                                nc.tensor.matmul(
                                    pt[q * 64 : (q + 1) * 64,
                                       lt * ncoln : (lt + 1) * ncoln],
                                    wlt[:, ln * 128 + x * 64 :
                                        ln * 128 + (x + 1) * 64],
                                    rhs,
                                    start=(x == 0), stop=(x == 1),
                                    skip_group_check=True,
                                    tile_position=(0, q * 64),
                                )
                    # batched 2-pass evict: add broadcast bias, then relu
                    tmp = tpool.tile([128, TCOL], BF16, tag="etmp",
                                     name=f"t{lvl}_{t0}")
                    bias_ap = bias_t[lvl][:, t0 : t0 + gper].unsqueeze(2) \
                        .broadcast_to([128, gper, ncoln])
                    ptv = pt[:].rearrange("p (n c) -> p n c", n=gper)
                    tv = tmp[:, : gper * ncoln].rearrange(
                        "p (n c) -> p n c", n=gper)
                    nc.vector.tensor_tensor(tv, ptv, bias_ap, op=ALU.add)
                    nc.scalar.activation(
                        nxtv[:, t0 : t0 + gper, :],
                        tv, AF.Relu,
                    )
            cur = nxt

        # ---------------- level 6 (node pairs -> F [128=(s,c),(pair,b)]) -----
        F = fpool.tile([128, 32 * BG], BF16, tag="feats", name="f6")
        Fv = F[:].rearrange("p (n b) -> p n b", n=32)
        curv = cur[:].rearrange("p (n b h) -> p n b h", n=64, b=BG)
        pt6 = ppool.tile([128, 32 * BG], F32, tag="ps",
                         padded_shape=[128, TCOL], name="p6")
        for g0 in range(0, 64, 16):
            w6t = wpool.tile([128, 16 * 128], BF16, tag="wch", name=f"w6_{g0}")
            nc.sync.dma_start(
                w6t[:], p["w6"][:, g0 * 128 : (g0 + 16) * 128]
            )
            for node in range(g0, g0 + 16):
                pr, s = node // 2, node % 2
                ln = node - g0
                for x in (0, 1):
                    rhs = curv[:, node, :, x]
                    nc.tensor.matmul(
                        pt6[s * 64 : (s + 1) * 64, pr * BG : (pr + 1) * BG],
                        w6t[:, ln * 128 + x * 64 : ln * 128 + (x + 1) * 64],
                        rhs,
                        start=(x == 0), stop=(x == 1),
                        skip_group_check=True,
                        tile_position=(0, s * 64),
                    )
        tmp6 = tpool.tile([128, 32 * BG], BF16, tag="etmp", name="t6")
        nc.vector.tensor_tensor(tmp6[:], pt6[:], b6bc_t[:], op=ALU.add)
        nc.scalar.activation(F[:], tmp6[:], AF.Relu)

        # ---------------- dense (row-tiled K=64 per s) ----------------
        t2s = fpool.tile([128, NK * NK * BG], F32, tag="t2s", name="t2s")
        t2sv = t2s[:].rearrange("m (n b) -> m n b", n=NK * NK)
        wdt = const.tile([128, 32 * 128], BF16, name="wd")
        nc.sync.dma_start(wdt[:], p["wd"][:])
        ptd = {}
        for s in (0, 1):
            ptd[s] = ppool.tile([128, 32 * BG], F32, tag="ps",
                                padded_shape=[128, TCOL], name=f"pd{s}")
        for pr in range(32):
            for s in (0, 1):
                nc.tensor.matmul(
                    ptd[s][:, pr * BG : (pr + 1) * BG],
                    wdt[s * 64 : (s + 1) * 64, pr * 128 : (pr + 1) * 128],
                    Fv[s * 64 : (s + 1) * 64, pr, :],
                    start=True, stop=True,
                    tile_position=(s * 64, 0),
                )
        for s in (0, 1):
            if s == 0:
                nc.vector.tensor_copy(t2sv[:, s::2, :], ptd[s][:].rearrange(
                    "m (n b) -> m n b", n=32))
            else:
                nc.scalar.copy(t2sv[:, s::2, :], ptd[s][:].rearrange(
                    "m (n b) -> m n b", n=32))
        nc.sync.dma_start(t2[:], t2s[:])
    nc.compile()
    return nc


# ----------------------------------------------------------------------------
# entry point
# ----------------------------------------------------------------------------

def kernel(**inputs):
    inputs = {k: np.asarray(v) for k, v in inputs.items()}
    wblobs = _prep_weights(inputs)
    nc = _build_kernel()
    in_maps = []
    for c in range(NCORES):
        m = dict(wblobs)
        m["a0"] = _prep_input(inputs["in_data"][c * BC : (c + 1) * BC])
        in_maps.append(m)
    res = run_bass_kernel_spmd(nc, in_maps, list(range(NCORES)))
    outs = [_decode_output(res.results[c]["t2"]) for c in range(NCORES)]
    return np.concatenate(outs, axis=0).astype(np.float32)


if __name__ == "__main__":
    import reference as ref

    inputs = {k: np.asarray(v) for k, v in ref.setup_inputs().items()}
    expected = np.asarray(ref.reference(**inputs))
    actual = kernel(**inputs)
    err = np.abs(actual - expected).max()
    rel = err / np.abs(expected).max()
    print("absmax:", err, "rel:", rel)


# revision 11
# speedup vs baseline: 1.8053x; 1.0717x over previous
"""Trainium2 Bass kernel for nn_ButterflyLayer2D (butterfly 2D CNN).

Strategy: pure data parallel over 8 NeuronCores (16 batch each).

Layouts (per core, bf16 activations):
  - conv-level inputs live in SBUF as [128 = (w%2)*64 + c, (node, b, h, w2)]
    so each 2x2-stride-2 per-node conv is computed with the q-scheme:
    output-w-parity q lands in psum partition half q via col-masked M=64
    matmuls at tile_position (0, 64q) (the two q matmuls run concurrently
    on the PE), x (h-parity) accumulates in PSUM.  K = (y, ci) = 128.
  - psum tiles then map 1:1 onto the next level's layout: partition
    (q, co) -> (w%2, c), cols (b, ho, w2o) -> (b, h, w2).  Every eviction
    is a single full-width 128-partition op with contiguous reads and
    writes (relu+bias fused via ScalarE activation / VectorE tensor_scalar,
    alternating engines).
  - deep levels (4, 5) batch many nodes into one [128, 1024] psum tile and
    evict with a 2-pass batched op (tensor_tensor add with a broadcast
    per-node bias AP, then relu) to amortize per-op overheads.
  - level 6 packs node pairs: psum [128=(s, c), (pair, b)]; dense reads the
    resulting F [128=(s, c), (pair, b)] with row-tiled K=64 matmuls
    (tile_position (64s, 0)) writing [128=(r,ou,ov), (pair, b)] per s.
  - the input 4x4-patch conv uses K=16 row-groups spread over 4 partition
    groups (one per b%4) x 2 col groups for 8-way PE tile concurrency.
  - ~20 warm-up matmuls on a memset tile run at t=0 (concurrent with input
    DMA) so the PE HAM clock-gate reaches 8/8 before the real work starts.
Weights are streamed from HBM in chunks through recycled tile tags; DMAs
are issued in consumption order with the input staged first.
"""

import numpy as np
from contextlib import ExitStack

import concourse.bass as bass
import concourse.tile as tile
from concourse import bacc, mybir
from concourse.bass_utils import run_bass_kernel_spmd

F32 = mybir.dt.float32
BF16 = mybir.dt.bfloat16
AF = mybir.ActivationFunctionType
ALU = mybir.AluOpType

B, IN, NLVL, KLVL, C = 128, 256, 6, 3, 64
TCOL = 1024               # psum tile columns
PBUFS = 4                 # psum tile bufs
NK, OU, OV = 8, 8, 8
NCORES = 8
BC = B // NCORES          # 16 per-core batch
PH = 1                    # phases per core
BG = BC // PH             # batch per phase
HALF = BG // 4            # input-conv b-subgroups per partition group
LVL_NODES = [4, 16, 64, 64, 64, 64]          # nodes per level
LVL_HIN = [64, 32, 16, 8, 4, 2]              # spatial H into each level
NWARM = 28                # HAM warm-up matmuls


# ----------------------------------------------------------------------------
# host-side pre-arrangement
# ----------------------------------------------------------------------------

def _prep_weights(inputs):
    """Weights/biases blobs shared by all cores."""
    import ml_dtypes
    out = {}
    # input filter: lhsT [16=(p,q), 64], replicated at partition bases 0/32/64/96
    fin = inputs["in_filter"][:, :, 0, :].reshape(16, C).astype(np.float32)
    finr = np.zeros((128, C), np.float32)
    for g in range(4):
        finr[g * 32 : g * 32 + 16] = fin
    out["fin"] = finr.astype(ml_dtypes.bfloat16)
    out["bin"] = np.concatenate([inputs["in_bias"], inputs["in_bias"]]).reshape(
        128, 1
    ).astype(np.float32)

    for lvl in range(1, NLVL + 1):
        f = inputs[f"f{lvl}"].astype(np.float32)  # [n,n,2,2,C,C] (x,y,ci,co)
        n = f.shape[0]
        n2 = n * n
        # per node lhsT [(y*64+ci), (x*64+co)], node-major blob
        w = f.transpose(0, 1, 3, 4, 2, 5).reshape(n2, 2 * C, 2 * C)
        out[f"w{lvl}"] = np.ascontiguousarray(w.transpose(1, 0, 2)).reshape(
            128, n2 * 128
        ).astype(ml_dtypes.bfloat16)
        b = inputs[f"b{lvl}"].astype(np.float32).reshape(n2, C)
        if lvl < NLVL:
            # [128, nodes]: rows (q,c) with bias duplicated across q
            bb = np.concatenate([b, b], axis=1)  # [nodes, 128]
            out[f"b{lvl}"] = np.ascontiguousarray(bb.T)
        else:
            # level-6 bias broadcast blob [128=(s,c), (pair, b)]
            bb = b.reshape(n2 // 2, 2, C)            # [pair, s, c]
            bb = bb.transpose(1, 2, 0)               # [s, c, pair]
            bb = np.repeat(bb.reshape(128, n2 // 2, 1), BG, axis=2)
            out["b6bc"] = np.ascontiguousarray(
                bb.reshape(128, n2 // 2 * BG)
            ).astype(ml_dtypes.bfloat16)
    # dense: blob [128=(s*64+c), (pair, r*64+ou*8+ov)]
    wd = inputs["Wd"].astype(np.float32).reshape(NK * NK, 2, C, OU * OV)
    wd = wd.reshape(NK * NK // 2, 2, 2, C, OU * OV)   # [pair, s, r, c, k]
    wd = wd.transpose(1, 3, 0, 2, 4)                  # [s, c, pair, r, k]
    out["wd"] = np.ascontiguousarray(wd.reshape(128, NK * NK // 2 * 128)).astype(
        ml_dtypes.bfloat16
    )
    return out


def _prep_input(in_data_core):
    """Per-core input blob: [64 = (b%4)*16 + (i%4)*4 + (j%4),
    (ph, b//4%2, x=i//4, y4=j//4)] packed (no zero rows)."""
    import ml_dtypes
    ind = in_data_core[:, :, :, 0]  # [16, 256, 256]
    a = ind.reshape(PH, HALF, 4, 64, 4, 64, 4)  # [ph, half, g, x, p, y4, q]
    a = a.transpose(2, 4, 6, 0, 1, 3, 5)        # [g, p, q, ph, half, x, y4]
    return np.ascontiguousarray(a).reshape(64, PH * HALF * 64 * 64).astype(ml_dtypes.bfloat16)


def _decode_output(t2_core):
    """t2 [128=(r,ou,ov), (ph, node, bl)] -> [16, 64, 64, 2]."""
    t = t2_core.reshape(2, OU, OV, PH, NK, NK, BG)  # r,ou,ov,ph,u,v,bl
    t = t.transpose(3, 6, 4, 1, 5, 2, 0)            # ph,bl,u,ou,v,ov,r
    return np.ascontiguousarray(t).reshape(BC, NK * OU, NK * OV, 2)


# ----------------------------------------------------------------------------
# device kernel
# ----------------------------------------------------------------------------

def _build_kernel():
    nc = bacc.Bacc(None, target_bir_lowering=False)
    p = {}
    p["a0"] = nc.declare_dram_parameter("a0", [64, PH * HALF * 64 * 64], BF16, isOutput=False)
    p["fin"] = nc.declare_dram_parameter("fin", [128, C], BF16, isOutput=False)
    p["bin"] = nc.declare_dram_parameter("bin", [128, 1], F32, isOutput=False)
    for lvl in range(1, NLVL + 1):
        n2 = LVL_NODES[lvl - 1]
        p[f"w{lvl}"] = nc.declare_dram_parameter(f"w{lvl}", [128, n2 * 128], BF16, isOutput=False)
        if lvl < NLVL:
            p[f"b{lvl}"] = nc.declare_dram_parameter(f"b{lvl}", [128, n2], F32, isOutput=False)
    p["b6bc"] = nc.declare_dram_parameter("b6bc", [128, 32 * BG], BF16, isOutput=False)
    p["wd"] = nc.declare_dram_parameter("wd", [128, 32 * 128], BF16, isOutput=False)
    t2 = nc.declare_dram_parameter("t2", [128, PH * NK * NK * BG], F32, isOutput=True)

    evict_ctr = [0]

    def evict(out_ap, psum_ap, bias_ap):
        """relu(psum + bias) -> sbuf, alternating engines to split the load."""
        evict_ctr[0] += 1
        # ACT is 1.25x faster per element than DVE on TRN2: give it 5 of 9
        if evict_ctr[0] % 9 in (0, 2, 4, 6, 8):
            nc.scalar.activation(out_ap, psum_ap, AF.Relu, bias=bias_ap)
        else:
            nc.vector.tensor_scalar(out_ap, psum_ap, bias_ap, 0.0,
                                    op0=ALU.add, op1=ALU.max)

    with tile.TileContext(nc) as tc, ExitStack() as ctx:
        const = ctx.enter_context(tc.tile_pool(name="const", bufs=1))
        wpool = ctx.enter_context(tc.tile_pool(name="wts", bufs=5))
        apool = ctx.enter_context(tc.tile_pool(name="acts", bufs=1))
        inpool = ctx.enter_context(tc.tile_pool(name="inp", bufs=1))
        fpool = ctx.enter_context(tc.tile_pool(name="feat", bufs=1))
        tpool = ctx.enter_context(tc.tile_pool(name="tmp", bufs=2))
        ppool = ctx.enter_context(tc.tile_pool(name="ps", bufs=PBUFS, space="PSUM"))

        # ------------- HAM warm-up (runs while input DMA streams) -------------
        wm = const.tile([128, 256], BF16, name="warm")
        nc.vector.memset(wm[:], 0.0)
        for i in range(NWARM):
            ptw = ppool.tile([128, 256], F32, tag="ps", padded_shape=[128, TCOL],
                             name=f"warm{i}")
            nc.tensor.matmul(ptw[:], wm[:, 0:128], wm[:],
                             start=True, stop=True)

        # ------------- input + constant DMAs (consumption order) -------------
        fin_t = const.tile([128, C], BF16)
        nc.sync.dma_start(fin_t[:], p["fin"][:])
        bin_t = const.tile([128, 1], F32)
        nc.sync.dma_start(bin_t[:], p["bin"][:])
        a0s = inpool.tile([128, PH * HALF * 64 * 64], BF16, tag="a0s", name="a0s")
        for g in range(4):
            nc.sync.dma_start(
                a0s[g * 32 : g * 32 + 16, :],
                p["a0"][g * 16 : (g + 1) * 16, :],
            )
        bias_t = {}
        for lvl in range(1, NLVL):
            bias_t[lvl] = const.tile([128, LVL_NODES[lvl - 1]], F32,
                                     tag=f"bias{lvl}", name=f"bias{lvl}")
            nc.sync.dma_start(bias_t[lvl][:], p[f"b{lvl}"][:])
        b6bc_t = const.tile([128, 32 * BG], BF16, name="b6bc")
        nc.sync.dma_start(b6bc_t[:], p["b6bc"][:])
        wdt = const.tile([128, 32 * 128], BF16, name="wd")
        nc.sync.dma_start(wdt[:], p["wd"][:])

        ph = 0
        a0v = a0s[:].rearrange("p (h x y) -> p h x y", h=HALF, x=64)

        # ---------------- input conv ----------------
        # X slab: [128=(y%2,c), (b, h=64, w2=32)]
        X = apool.tile([128, BG * 64 * 32], BF16, tag="s0", name="x0")
        Xv = X[:].rearrange("p (b h w) -> p b h w", b=BG, h=64)
        # g-major order: only the g=0 input-DMA chunk gates the start
        for bl in [g + 4 * h for g in range(4) for h in range(4)]:
            g, half = bl % 4, bl // 4
            for xh in range(2):
                pt = ppool.tile([128, TCOL], F32, tag="ps",
                                padded_shape=[128, TCOL],
                                name=f"pin{bl}_{xh}")
                for sub in range(2):
                    xq = xh * 2 + sub
                    for q in (0, 1):
                        rhs = a0v[g * 32 : g * 32 + 16, half,
                                  xq * 16 : (xq + 1) * 16, q::2]
                        nc.tensor.matmul(
                            pt[q * 64 : (q + 1) * 64,
                               sub * 512 : (sub + 1) * 512],
                            fin_t[g * 32 : g * 32 + 16, :],
                            rhs,
                            start=True, stop=True,
                            tile_position=(g * 32, q * 64),
                        )
                evict(Xv[:, bl, xh * 32 : (xh + 1) * 32, :], pt[:], bin_t[:, 0:1])

        # ---------------- levels 1..3 (q-scheme, per-node psum) --------------
        cur, cur_nodes = X, 1
        tags = ["s1", "s0", "s1"]
        for lvl in (1, 2, 3):
            n2 = LVL_NODES[lvl - 1]
            grid = int(np.sqrt(n2))
            pgrid = int(np.sqrt(cur_nodes))
            Hin = LVL_HIN[lvl - 1]
            W2in = Hin // 2
            Ho, Ko = Hin // 2, W2in // 2      # psum cols per b = Ho*Ko
            ncolb = Ho * Ko
            bper = min(BG, TCOL // ncolb)
            nsub = (bper * ncolb) // 512       # 512-col chunks per tile
            bsub = bper // nsub
            nxt = apool.tile([128, n2 * BG * ncolb], BF16,
                             tag=tags[lvl - 1], name=f"a{lvl}")
            curv = cur[:].rearrange("p (n b h w) -> p n b h w",
                                    n=cur_nodes, b=BG, h=Hin)
            nxtv = nxt[:].rearrange("p (n b h w) -> p n b h w",
                                    n=n2, b=BG, h=Ho)
            # stream this level's weights in one or two chunks
            wchunk = min(n2, 16)
            for g0 in range(0, n2, wchunk):
                wlt = wpool.tile([128, 16 * 128], BF16, tag="wch",
                                 name=f"w{lvl}_{g0}")
                nc.sync.dma_start(
                    wlt[:, : wchunk * 128],
                    p[f"w{lvl}"][:, g0 * 128 : (g0 + wchunk) * 128],
                )
                for node in range(g0, g0 + wchunk):
                    u, v = node // grid, node % grid
                    pn = (u // 2) * pgrid + (v // 2)
                    ln = node - g0
                    for bs in range(0, BG, bper):
                        pt = ppool.tile([128, bper * ncolb], F32, tag="ps",
                                        padded_shape=[128, TCOL],
                                        name=f"p{lvl}_{node}_{bs}")
                        for x in (0, 1):
                            for q in (0, 1):
                                for sb in range(nsub):
                                    b1 = bs + sb * bsub
                                    rhs = curv[:, pn, b1 : b1 + bsub,
                                               x::2, q::2]
                                    nc.tensor.matmul(
                                        pt[q * 64 : (q + 1) * 64,
                                           sb * 512 : (sb + 1) * 512],
                                        wlt[:, ln * 128 + x * 64 :
                                            ln * 128 + (x + 1) * 64],
                                        rhs,
                                        start=(x == 0), stop=(x == 1),
                                        skip_group_check=True,
                                        tile_position=(0, q * 64),
                                    )
                        evict(
                            nxtv[:, node, bs : bs + bper, :, :],
                            pt[:],
                            bias_t[lvl][:, node : node + 1],
                        )
            cur, cur_nodes = nxt, n2

        # ---------------- levels 4..5 (q-scheme, node-batched psum) ----------
        for lvl in (4, 5):
            n2 = 64
            Hin = LVL_HIN[lvl - 1]
            W2in = Hin // 2
            Ho, Ko = Hin // 2, W2in // 2
            ncoln = BG * Ho * max(Ko, 1)       # cols per node (Ko>=1)
            gper = min(TCOL // ncoln, 16)      # nodes per psum tile
            nxt = apool.tile([128, n2 * ncoln], BF16,
                             tag=("s0" if lvl == 4 else "s1"), name=f"a{lvl}")
            curv = cur[:].rearrange("p (n b h w) -> p n b h w",
                                    n=64, b=BG, h=Hin)
            nxtv = nxt[:].rearrange("p (n c) -> p n c", n=n2)
            for g0 in range(0, n2, 16):
                wlt = wpool.tile([128, 16 * 128], BF16, tag="wch",
                                 name=f"w{lvl}_{g0}")
                nc.sync.dma_start(
                    wlt[:], p[f"w{lvl}"][:, g0 * 128 : (g0 + 16) * 128]
                )
                for t0 in range(g0, g0 + 16, gper):
                    pt = ppool.tile([128, gper * ncoln], F32, tag="ps",
                                    padded_shape=[128, TCOL],
                                    name=f"p{lvl}_{t0}")
                    for node in range(t0, t0 + gper):
                        ln, lt = node - g0, node - t0
                        for x in (0, 1):
                            for q in (0, 1):
                                rhs = curv[:, node, :, x::2, q::2]
                                nc.tensor.matmul(
                                    pt[q * 64 : (q + 1) * 64,
                                       lt * ncoln : (lt + 1) * ncoln],
                                    wlt[:, ln * 128 + x * 64 :
                                        ln * 128 + (x + 1) * 64],
                                    rhs,
                                    start=(x == 0), stop=(x == 1),
                                    skip_group_check=True,
                                    tile_position=(0, q * 64),
                                )
                    # batched 2-pass evict: add broadcast bias, then relu
                    tmp = tpool.tile([128, TCOL], BF16, tag="etmp",
                                     name=f"t{lvl}_{t0}")
                    bias_ap = bias_t[lvl][:, t0 : t0 + gper].unsqueeze(2) \
                        .broadcast_to([128, gper, ncoln])
                    ptv = pt[:].rearrange("p (n c) -> p n c", n=gper)
                    tv = tmp[:, : gper * ncoln].rearrange(
                        "p (n c) -> p n c", n=gper)
                    nc.vector.tensor_tensor(tv, ptv, bias_ap, op=ALU.add)
                    nc.scalar.activation(
                        nxtv[:, t0 : t0 + gper, :],
                        tv, AF.Relu,
                    )
            cur = nxt

        # ---------------- level 6 (node pairs -> F [128=(s,c),(pair,b)]) -----
        F = fpool.tile([128, 32 * BG], BF16, tag="feats", name="f6")
        Fv = F[:].rearrange("p (n b) -> p n b", n=32)
        curv = cur[:].rearrange("p (n b h) -> p n b h", n=64, b=BG)
        pt6 = ppool.tile([128, 32 * BG], F32, tag="ps",
                         padded_shape=[128, TCOL], name="p6")
        for g0 in range(0, 64, 16):
            w6t = wpool.tile([128, 16 * 128], BF16, tag="wch", name=f"w6_{g0}")
            nc.sync.dma_start(
                w6t[:], p["w6"][:, g0 * 128 : (g0 + 16) * 128]
            )
            for node in range(g0, g0 + 16):
                pr, s = node // 2, node % 2
                ln = node - g0
                for x in (0, 1):
                    rhs = curv[:, node, :, x]
                    nc.tensor.matmul(
                        pt6[s * 64 : (s + 1) * 64, pr * BG : (pr + 1) * BG],
                        w6t[:, ln * 128 + x * 64 : ln * 128 + (x + 1) * 64],
                        rhs,
                        start=(x == 0), stop=(x == 1),
                        skip_group_check=True,
                        tile_position=(0, s * 64),
                    )
        tmp6 = tpool.tile([128, 32 * BG], BF16, tag="etmp", name="t6")
        nc.vector.tensor_tensor(tmp6[:], pt6[:], b6bc_t[:], op=ALU.add)
        nc.scalar.activation(F[:], tmp6[:], AF.Relu)

        # ---------------- dense (row-tiled K=64 per s) ----------------
        t2s = fpool.tile([128, NK * NK * BG], F32, tag="t2s", name="t2s")
        t2sv = t2s[:].rearrange("m (n b) -> m n b", n=NK * NK)
        ptd = {}
        for s in (0, 1):
            ptd[s] = ppool.tile([128, 32 * BG], F32, tag="ps",
                                padded_shape=[128, TCOL], name=f"pd{s}")
        for pr in range(32):
            for s in (0, 1):
                nc.tensor.matmul(
                    ptd[s][:, pr * BG : (pr + 1) * BG],
                    wdt[s * 64 : (s + 1) * 64, pr * 128 : (pr + 1) * 128],
                    Fv[s * 64 : (s + 1) * 64, pr, :],
                    start=True, stop=True,
                    tile_position=(s * 64, 0),
                )
        for s in (0, 1):
            if s == 0:
                nc.vector.tensor_copy(t2sv[:, s::2, :], ptd[s][:].rearrange(
                    "m (n b) -> m n b", n=32))
            else:
                nc.scalar.copy(t2sv[:, s::2, :], ptd[s][:].rearrange(
                    "m (n b) -> m n b", n=32))
        nc.sync.dma_start(t2[:], t2s[:])
    nc.compile()
    return nc


# ----------------------------------------------------------------------------
# entry point
# ----------------------------------------------------------------------------

def kernel(**inputs):
    inputs = {k: np.asarray(v) for k, v in inputs.items()}
    wblobs = _prep_weights(inputs)
    nc = _build_kernel()
    in_maps = []
    for c in range(NCORES):
        m = dict(wblobs)
        m["a0"] = _prep_input(inputs["in_data"][c * BC : (c + 1) * BC])
        in_maps.append(m)
    res = run_bass_kernel_spmd(nc, in_maps, list(range(NCORES)))
    outs = [_decode_output(res.results[c]["t2"]) for c in range(NCORES)]
    return np.concatenate(outs, axis=0).astype(np.float32)


if __name__ == "__main__":
    import reference as ref

    inputs = {k: np.asarray(v) for k, v in ref.setup_inputs().items()}
    expected = np.asarray(ref.reference(**inputs))
    actual = kernel(**inputs)
    err = np.abs(actual - expected).max()
    rel = err / np.abs(expected).max()
    print("absmax:", err, "rel:", rel)
